# revision 35
# baseline (speedup 1.0000x reference)
"""Trainium2 Bass kernel for Llama GQA attention (B=1, S=2048, HID=4096,
NH=32, NKV=8, HD=128), tensor-parallel over 8 NeuronCores.

Core c owns Q heads [4c, 4c+4) and KV head c (one GQA group per core).
Each core computes its partial contribution to out = attn_out @ wo (wo is
sharded on its input dim); the host sums the 8 partials.

v2 over the 472us baseline:
- all DRAM inputs pre-laid-out on the host so every DMA reads >=8KB
  contiguous per partition (was 256B-1KB packets; startup was DMA-bound)
- phase-2 weights (wo, mask) + cos/sin loaded during phase 1; phase-2
  SBUF pools reuse phase-1 space with no pool-exit barrier on the PE path
- ps_s (QK scores PSUM) double-buffered so block i+1's QK matmuls overlap
  block i's exp on the ScalarE (was a 1.2us serialization per pair)
- causal narrowing: diagonal 128-col k-tiles only compute q columns
  >= tile offset (saves ~37% of diagonal QK/exp/attnV work)
- softmax denominator via GpSimd partition-reduce + partition-broadcast
  (was: PE ones-matmul + round-trip DMA bounce through DRAM)
- output written fp16 (host sums partials in fp32)
"""

import sys

sys.path.insert(0, "/opt/trn_rl_repo")

import numpy as np

P = 128
S = 2048
HID = 4096
HPC = 4          # q heads per core
NCORES = 8
KT = HID // P    # 32 k-tiles over hidden dim
KQ = 8           # k-tiles per xT chunk
NKQ = KT // KQ   # 4 chunks
SBLK = S // 512  # 4 blocks of 512 along sequence
ST = S // P      # 16 s-tiles of 128
SCALING = 128 ** -0.5
EXP_BIAS = -2.0  # constant shift inside exp; cancels in normalization


def _build_nc():
    import concourse.bass as bass
    import concourse.mybir as mybir
    import concourse.tile as tile
    from concourse import bacc
    from concourse import bass_isa
    from concourse.masks import make_identity

    f16 = mybir.dt.float16
    f32 = mybir.dt.float32
    Alu = mybir.AluOpType
    Act = mybir.ActivationFunctionType

    nc = bacc.Bacc(
        "TRN2",
        target_bir_lowering=False,
        debug=False,
        enable_asserts=False,
        num_devices=NCORES,
    )

    xT_d = nc.dram_tensor("xT", [P, SBLK, KT, 512], f16, kind="ExternalInput")
    wq_d = nc.dram_tensor("wq", [P, KT, HPC * P], f16, kind="ExternalInput")
    wk_d = nc.dram_tensor("wk", [P, KT, P], f16, kind="ExternalInput")
    wv_d = nc.dram_tensor("wv", [P, KT, P], f16, kind="ExternalInput")
    wo_d = nc.dram_tensor("wo", [P, HPC, HID], f16, kind="ExternalInput")
    cos_d = nc.dram_tensor("cosT", [P, S], f16, kind="ExternalInput")
    sin_d = nc.dram_tensor("sinS", [P, S], f16, kind="ExternalInput")
    tri_d = nc.dram_tensor("tri", [P, P], f16, kind="ExternalInput")
    out_d = nc.dram_tensor("out", [S, HID], f16, kind="ExternalOutput")

    with tile.TileContext(nc) as tc:
        with tc.tile_pool(name="const", bufs=1) as constp:
            qTs = [[constp.tile([P, 512], f16, tag=f"qT{h}_{sb}", name=f"qT{h}_{sb}")
                    for sb in range(SBLK)] for h in range(HPC)]
            kTs = [constp.tile([P, 512], f16, tag=f"kT{sb}", name=f"kT{sb}")
                   for sb in range(SBLK)]
            Vts = [constp.tile([P, P], f16, tag=f"Vt{st}", name=f"Vt{st}")
                   for st in range(ST)]
            OTs = [constp.tile([P, 512], f16, tag=f"OT{i}", name=f"OT{i}")
                   for i in range(HPC * SBLK)]
            ident = constp.tile([P, P], f16, tag="ident")
            make_identity(nc, ident[:])
            ebias = constp.tile([P, 1], f32, tag="ebias")
            nc.vector.memset(ebias[:], EXP_BIAS)
            ones16 = constp.tile([P, 1], f16, tag="o16")
            nc.vector.memset(ones16[:], 1.0)
            cos_sb = constp.tile([P, S], f16, tag="cos")
            sin_sb = constp.tile([P, S], f16, tag="sin")
            tri_sb = constp.tile([P, P], f16, tag="tri")
            wo_sb = constp.tile([P, HPC, HID], f16, tag="wo")

            # ---- phase 1: Q/K/V projections (+ RoPE on q, k) ----
            with (
                tc.tile_pool(name="p1w", bufs=1) as p1w,
                tc.tile_pool(name="xt", bufs=4) as xtp,
                tc.tile_pool(name="rope", bufs=3) as ropep,
                tc.tile_pool(name="ps1", bufs=1, space="PSUM") as ps1,
            ):
                # PSUM bank assignment order matters: phase 2's ps_s lands on
                # the first banks allocated here, so allocate the chains that
                # are read earliest at the phase boundary (k, v, transposes)
                # first, and rope in matching order (k first, q0 last; q0 is
                # double-buffered for the sb-boundary handoff).
                ps_warm = ps1.tile([P, 512], f32, tag="psvt", name="warm")
                for _ in range(24):
                    nc.tensor.matmul(ps_warm[:, 0:P], ident[:], ident[:],
                                     start=True, stop=True)

                # startup-critical loads split in halves across four queues;
                # everything else (cos/sin/tri/wo/wq1-3) deferred to kq=1
                wq_sb = [p1w.tile([P, KQ, HPC * P], f16, tag=f"wq{kq}", name=f"wq{kq}")
                         for kq in range(NKQ)]
                nc.scalar.dma_start(wq_sb[0][:, 0:KQ // 2, :], wq_d.ap()[:, 0:KQ // 2, :])
                nc.scalar.dma_start(
                    wq_sb[0][:, KQ // 2:KQ, :], wq_d.ap()[:, KQ // 2:KQ, :]
                )
                wk_sb = p1w.tile([P, KT, P], f16, tag="wk")
                wv_sb = p1w.tile([P, KT, P], f16, tag="wv")
                HK = KT // 2
                nc.gpsimd.dma_start(wk_sb[:, 0:HK, :], wk_d.ap()[:, 0:HK, :])
                nc.gpsimd.dma_start(wv_sb[:, 0:HK, :], wv_d.ap()[:, 0:HK, :])

                def rope(p16, out, scol):
                    """out(f16) = p16*cos + rotate_half(p16)*sin; the sign of
                    rotate_half is folded into sinS on the host. p16 is the
                    fp16 SBUF copy of the projection (the PSUM bank was
                    already released by the ScalarE evacuation copy)."""
                    c = cos_sb[:, scol]
                    sn = sin_sb[:, scol]
                    t1 = ropep.tile([P, 512], f16, tag="t1")
                    nc.vector.tensor_tensor(t1[:], p16[:], c, Alu.mult)
                    t2 = ropep.tile([P, 512], f16, tag="t2")
                    # rotate_half: lanes 0:64 read partitions 64:128, vice
                    # versa; sinS is pre-rolled by 64 partitions on the host so
                    # both SBUF operands share a base partition (NCC_IBIR297)
                    nc.vector.tensor_tensor(t2[0:64, :], p16[64:128, :], sn[64:128, :], Alu.mult)
                    nc.vector.tensor_tensor(t2[64:128, :], p16[0:64, :], sn[0:64, :], Alu.mult)
                    nc.vector.tensor_tensor(out, t1[:], t2[:], Alu.add)

                for sb in range(SBLK):
                    scol = slice(sb * 512, (sb + 1) * 512)
                    # allocation (=bank) order: psvt, pstr(warm above), psq0..:
                    # phase 2's ps_s reuses the first 4 banks, which are the
                    # ones freed soonest after sb=3 (vts copy, transposes,
                    # psq0's idle second buffer, rope q0)
                    ps_vt = ps1.tile([P, 512], f32, tag="psvt")
                    ps_q = [ps1.tile([P, 512], f32, tag=f"psq{h}", name=f"ps_q{h}",
                                     bufs=(2 if h == 0 else 1))
                            for h in range(HPC)]
                    ps_k = ps1.tile([P, 512], f32, tag="psk")
                    for kq in range(NKQ):
                        # xt chunks alternate sync/gpsimd queues; each queue
                        # tops out ~130GB/s, and the early loads nearly
                        # saturate aggregate HBM, so just-in-time order matters
                        xq = nc.sync if kq % 2 == 0 else nc.gpsimd
                        xt = xtp.tile([P, KQ, 512], f16, tag="xt", name=f"xt{kq}")
                        if sb == 0 and kq == 0:
                            nc.sync.dma_start(
                                xt[:, 0:KQ // 2, :],
                                xT_d.ap()[:, sb, 0:KQ // 2, :],
                            )
                            nc.sync.dma_start(
                                xt[:, KQ // 2:KQ, :],
                                xT_d.ap()[:, sb, KQ // 2:KQ, :],
                            )
                        else:
                            xq.dma_start(
                                xt[:], xT_d.ap()[:, sb, kq * KQ:(kq + 1) * KQ, :]
                            )
                        if sb == 0 and kq == 1:
                            # second halves of wk/wv land just before k-tile 16
                            nc.gpsimd.dma_start(wk_sb[:, HK:KT, :], wk_d.ap()[:, HK:KT, :])
                            nc.gpsimd.dma_start(wv_sb[:, HK:KT, :], wv_d.ap()[:, HK:KT, :])
                        if sb == 0 and kq == 3:
                            # scalar queue is idle after the wq chunks
                            nc.scalar.dma_start(cos_sb[:], cos_d.ap())
                            nc.scalar.dma_start(sin_sb[:], sin_d.ap())
                            nc.scalar.dma_start(tri_sb[:], tri_d.ap())
                        if sb == 1 and kq == 0:
                            # deferred past sb=0's HBM crunch; needed ~170us out
                            nc.scalar.dma_start(wo_sb[:], wo_d.ap())
                        if sb == 0 and kq < NKQ - 1:
                            nc.scalar.dma_start(
                                wq_sb[kq + 1][:], wq_d.ap()[:, (kq + 1) * KQ:(kq + 2) * KQ, :]
                            )
                        for k in range(KQ):
                            kg = kq * KQ + k
                            st, sp = kg == 0, kg == KT - 1
                            for h in range(HPC):
                                nc.tensor.matmul(
                                    ps_q[h][:],
                                    wq_sb[kq][:, k, h * P:(h + 1) * P],
                                    xt[:, k, :],
                                    start=st, stop=sp,
                                )
                            nc.tensor.matmul(
                                ps_k[:], wk_sb[:, kg, :], xt[:, k, :],
                                start=st, stop=sp,
                            )
                            nc.tensor.matmul(
                                ps_vt[:], wv_sb[:, kg, :], xt[:, k, :],
                                start=st, stop=sp,
                            )
                    # Evacuate all six PSUM chains to fp16 SBUF on the
                    # ScalarE (idle in phase 1): frees the PSUM banks within
                    # ~4us of the last matmul instead of after the full DVE
                    # rope chain (~12us), so the next sb / phase 2 start fast.
                    vts = ropep.tile([P, 512], f16, tag="vts")
                    nc.scalar.copy(vts[:], ps_vt[:])
                    q16 = [ropep.tile([P, 512], f16, tag=f"q16_{h}", name=f"q16_{h}")
                           for h in range(HPC)]
                    for h in range(HPC):
                        nc.scalar.copy(q16[h][:], ps_q[h][:])
                    k16 = ropep.tile([P, 512], f16, tag="k16")
                    nc.scalar.copy(k16[:], ps_k[:])
                    # V^T [hd, s] -> V [s, hd] via PE transpose, ahead of the
                    # ropes in the PE FIFO (only waits on the vts copy)
                    for j in range(4):
                        ps_tr = ps1.tile([P, P], f16, tag="pstr")
                        nc.tensor.transpose(ps_tr[:], vts[:, j * P:(j + 1) * P], ident[:])
                        nc.vector.tensor_copy(Vts[sb * 4 + j][:], ps_tr[:])
                    for h in range(HPC):
                        rope(q16[h], qTs[h][sb][:], scol)
                    rope(k16, kTs[sb][:], scol)

            # ---- phase 2+3: attention fused with the output projection ----
            # Block (qb, h) covers q rows [qb*512,(qb+1)*512) of head h.
            # Per block, "units" of 2 k-tiles each: off-diagonal units are
            # full-width; the 2 diagonal units narrow each 128-wide k-tile's
            # matmul/exp/attnV to q columns >= its offset (causal).
            with (
                tc.tile_pool(name="exp", bufs=16) as expp,
                tc.tile_pool(name="norm", bufs=2) as normp,
                tc.tile_pool(name="stage", bufs=3) as stagep,
                tc.tile_pool(name="dramscratch", bufs=3, space="DRAM") as dramp,
                tc.tile_pool(name="ps2", bufs=1, space="PSUM") as ps2,
            ):
                GORDER = [0, 1, 2, 3]
                blocks = [(qb, h) for qb in GORDER for h in range(HPC)]
                NBLK = len(blocks)

                pending = []          # normalize finishes deferred one block
                wo_queue = []         # (st, nt) output-projection chains
                stage_tiles = {}      # st -> [tile, ndone]
                copy_rr = [0]
                released = [0]        # count of groups whose wo chains are queued

                def finish(ent):
                    oti, ps_o, rec = ent
                    # broadcast 1/colsum over partitions via a DRAM bounce on
                    # the gpsimd queue (idle in phase 2; the sync queue's out
                    # writes would delay these latency-critical hops)
                    rdram = dramp.tile([1, 512], f32, tag="rdram", name="rdram")
                    nc.gpsimd.dma_start(rdram[:], rec[:])
                    bc = normp.tile([P, 512], f32, tag="bc", name="bc")
                    nc.gpsimd.dma_start(bc[:], rdram[:].to_broadcast((P, 512)))
                    nc.vector.tensor_tensor(OTs[oti][:], ps_o[:], bc[:], Alu.mult)

                def emit_wo_chain():
                    if not wo_queue:
                        return
                    st, nt = wo_queue.pop(0)
                    if st not in stage_tiles:
                        stage_tiles[st] = [
                            stagep.tile([P, HID], f16, tag="stage", name="stage"), 0
                        ]
                    stage, _ = stage_tiles[st]
                    ssl = slice((st % 4) * P, (st % 4 + 1) * P)
                    ps_w = ps2.tile([P, 512], f32, tag="ps_w", name="ps_w", bufs=2)
                    for h in range(HPC):
                        nc.tensor.matmul(
                            ps_w[:],
                            OTs[h * SBLK + st // 4][:, ssl],
                            wo_sb[:, h, nt * 512:(nt + 1) * 512],
                            start=(h == 0),
                            stop=(h == HPC - 1),
                        )
                    dst = stage[:, nt * 512:(nt + 1) * 512]
                    copy_rr[0] += 1
                    if copy_rr[0] % 2 == 0:  # 1:1 ACT:DVE balance
                        nc.scalar.copy(dst, ps_w[:])
                    else:
                        nc.vector.tensor_copy(dst, ps_w[:])
                    stage_tiles[st][1] += 1
                    if stage_tiles[st][1] == HID // 512:
                        nc.sync.dma_start(out_d.ap()[st * P:(st + 1) * P, :], stage[:])
                        del stage_tiles[st]

                def emit_unit(h, qb, g, acc):
                    """Unit g of block (qb,h): QK matmuls + exp + mask + acc
                    for k-tiles kb = 2g, 2g+1. Off-diagonal units (kb < 4qb)
                    are full-width single-exp; diagonal units narrow each
                    tile to q cols >= 128*j (j = kb - 4qb) and use the
                    shared [P,128] lower-triangle mask on the leading strip.
                    Returns (ex_tile, [(kb, exslice, width, coloff)])."""
                    ps_s = ps2.tile([P, 1024], f32, tag="ps_s", name="ps_s", bufs=2)
                    ex = expp.tile([P, 1024], f16, tag="ex", name="ex")
                    parts = []
                    diag = 2 * g >= 4 * qb
                    for t in range(2):
                        kb = 2 * g + t
                        j = kb - 4 * qb
                        off = j * P if diag else 0
                        w = 512 - off
                        sl = slice(t * 512 + off, (t + 1) * 512)
                        nc.tensor.matmul(
                            ps_s[:, sl],
                            kTs[kb // 4][:, (kb % 4) * P:(kb % 4 + 1) * P],
                            qTs[h][qb][:, off:512],
                            start=True,
                            stop=True,
                        )
                        parts.append((kb, sl, w, off))
                    if not diag:
                        nc.scalar.activation(
                            ex[:], ps_s[:], Act.Exp, bias=ebias[:], scale=SCALING
                        )
                    else:
                        for kb, sl, w, off in parts:
                            nc.scalar.activation(
                                ex[:, sl], ps_s[:, sl], Act.Exp,
                                bias=ebias[:], scale=SCALING,
                            )
                    for kb, sl, w, off in parts:
                        half = ex[:, sl]
                        if diag:
                            strip = ex[:, sl.start:sl.start + P]
                            nc.vector.tensor_tensor(strip, strip, tri_sb[:], Alu.mult)
                        if kb == 0:
                            nc.vector.tensor_copy(acc[:], half)
                        else:
                            nc.vector.tensor_tensor(
                                acc[:, off:512], acc[:, off:512], half, Alu.add
                            )
                    return ex, parts

                prev = None  # (units, nunit, ps_o, acc, oti)
                for i in range(NBLK + 1):
                    # wo chains for q-block group qb become eligible two
                    # blocks after the group's last block (normalize is
                    # deferred one block)
                    if i >= 6 and (i - 6) % 4 == 0:
                        qb_ready = GORDER[(i - 6) // 4]
                        released[0] = (i - 6) // 4 + 1
                        for st in range(qb_ready * 4, qb_ready * 4 + 4):
                            for nt in range(HID // 512):
                                wo_queue.append((st, nt))
                    if i < NBLK:
                        qb, h = blocks[i]
                        nunit = 2 * qb + 2
                        units = []
                        acc = normp.tile([P, 512], f16, tag="acc", name="acc", bufs=2)
                    else:
                        nunit = 0
                    if prev is not None:
                        punits, pnunit, ps_o, pacc, oti = prev
                    else:
                        pnunit = 0
                    for g in range(max(nunit, pnunit)):
                        if i < NBLK and g < nunit:
                            units.append(emit_unit(h, qb, g, acc))
                        if prev is not None and g < pnunit:
                            pex, pparts = punits[g]
                            last_kb = 2 * pnunit - 1
                            for kb, sl, w, off in pparts:
                                nc.tensor.matmul(
                                    ps_o[:, off:512], Vts[kb][:], pex[:, sl],
                                    start=(kb == 0), stop=(kb == last_kb),
                                )
                        # hold chains back near the end so the PE has wo work
                        # while the last blocks' normalizes complete
                        for _ in range(1 if i >= NBLK - 2 else 2):
                            emit_wo_chain()
                    if prev is not None:
                        # denominator: ones-vector matmul reduces acc over the
                        # partition axis; the colsum borrows a ps_w rotation
                        # slot so PSUM stays within 8 banks
                        ps_cs = ps2.tile([P, 512], f32, tag="ps_w", name="ps_cs",
                                         bufs=2)
                        nc.tensor.matmul(ps_cs[0:1, :], ones16[:], pacc[:],
                                         start=True, stop=True)
                        rec = normp.tile([1, 512], f32, tag="rec", name="rec",
                                         bufs=2)
                        nc.vector.reciprocal_approx_fast(rec[:], ps_cs[0:1, :])
                        if pending:
                            finish(pending.pop())
                        pending.append((oti, ps_o, rec))
                    if i < NBLK:
                        ps_o = ps2.tile([P, 512], f32, tag="ps_o", name="ps_o", bufs=2)
                        prev = (units, nunit, ps_o, acc, h * SBLK + qb)
                while pending:
                    finish(pending.pop())
                for j in range(released[0], SBLK):
                    qb = GORDER[j]
                    for st in range(qb * 4, qb * 4 + 4):
                        for nt in range(HID // 512):
                            wo_queue.append((st, nt))
                while wo_queue:
                    emit_wo_chain()

    nc.compile()
    return nc


_CACHE = {}


def _get_nc():
    if "nc" not in _CACHE:
        _CACHE["nc"] = _build_nc()
    return _CACHE["nc"]


def make_in_maps(hidden_states, cos, sin, wq, wk, wv, wo):
    x = np.asarray(hidden_states)[0].astype(np.float16)  # [S, HID]
    # xT_pre[p, sb, kt, c] = x[sb*512+c, kt*128+p]
    xT_pre = np.ascontiguousarray(
        x.reshape(SBLK, 512, KT, P).transpose(3, 0, 2, 1)
    )
    cosT = np.ascontiguousarray(np.asarray(cos)[0].T).astype(np.float16)
    sinT = np.ascontiguousarray(np.asarray(sin)[0].T).astype(np.float64)
    sinS = sinT.copy()
    sinS[:64] *= -1.0  # rotate_half sign fold: q'[d<64] -= q[d+64]*sin[d]
    # roll by 64 partitions: the kernel reads sn[64:128] against p16[64:128]
    # (and vice versa) to satisfy the equal-base-partition constraint
    sinS = np.concatenate([sinS[64:], sinS[:64]], axis=0)
    sinS = np.ascontiguousarray(sinS).astype(np.float16)
    f = np.arange(P)[None, :]
    p = np.arange(P)[:, None]
    tri = (f >= p).astype(np.float16)  # [128,128] lower-tri in col>=row sense
    wq = np.asarray(wq).astype(np.float16)
    wk = np.asarray(wk).astype(np.float16)
    wv = np.asarray(wv).astype(np.float16)
    wo = np.asarray(wo).astype(np.float16)

    in_maps = []
    for c in range(NCORES):
        wq_c = wq[:, c * 512:(c + 1) * 512]    # [4096, 512]
        wk_c = wk[:, c * P:(c + 1) * P]        # [4096, 128]
        wv_c = wv[:, c * P:(c + 1) * P]
        wo_c = wo[c * 512:(c + 1) * 512, :]    # [512, 4096]
        in_maps.append(
            {
                "xT": xT_pre,
                "wq": np.ascontiguousarray(
                    wq_c.reshape(KT, P, 512).transpose(1, 0, 2)
                ),
                "wk": np.ascontiguousarray(
                    wk_c.reshape(KT, P, P).transpose(1, 0, 2)
                ),
                "wv": np.ascontiguousarray(
                    wv_c.reshape(KT, P, P).transpose(1, 0, 2)
                ),
                "wo": np.ascontiguousarray(
                    wo_c.reshape(HPC, P, HID).transpose(1, 0, 2)
                ),
                "cosT": cosT,
                "sinS": sinS,
                "tri": tri,
            }
        )
    return in_maps


def run(in_maps, trace=False, **kw):
    from concourse.bass_utils import run_bass_kernel_spmd

    nc = _get_nc()
    return run_bass_kernel_spmd(
        nc, in_maps, core_ids=list(range(NCORES)), trace=trace, **kw
    )


def kernel(hidden_states, cos, sin, attn_mask, wq, wk, wv, wo):
    in_maps = make_in_maps(hidden_states, cos, sin, wq, wk, wv, wo)
    res = run(in_maps)
    parts = np.stack([np.asarray(r["out"], dtype=np.float32) for r in res.results])
    out = parts.sum(axis=0, dtype=np.float64).astype(np.float32)
    return out.reshape(1, S, HID)


# revision 40
# speedup vs baseline: 1.0136x; 1.0136x over previous
"""Trainium2 Bass kernel for Llama GQA attention (B=1, S=2048, HID=4096,
NH=32, NKV=8, HD=128), tensor-parallel over 8 NeuronCores.

Core c owns Q heads [4c, 4c+4) and KV head c (one GQA group per core).
Each core computes its partial contribution to out = attn_out @ wo (wo is
sharded on its input dim); the host sums the 8 partials.

v2 over the 472us baseline:
- all DRAM inputs pre-laid-out on the host so every DMA reads >=8KB
  contiguous per partition (was 256B-1KB packets; startup was DMA-bound)
- phase-2 weights (wo, mask) + cos/sin loaded during phase 1; phase-2
  SBUF pools reuse phase-1 space with no pool-exit barrier on the PE path
- ps_s (QK scores PSUM) double-buffered so block i+1's QK matmuls overlap
  block i's exp on the ScalarE (was a 1.2us serialization per pair)
- causal narrowing: diagonal 128-col k-tiles only compute q columns
  >= tile offset (saves ~37% of diagonal QK/exp/attnV work)
- softmax denominator via GpSimd partition-reduce + partition-broadcast
  (was: PE ones-matmul + round-trip DMA bounce through DRAM)
- output written fp16 (host sums partials in fp32)
"""

import sys

sys.path.insert(0, "/opt/trn_rl_repo")

import numpy as np

P = 128
S = 2048
HID = 4096
HPC = 4          # q heads per core
NCORES = 8
KT = HID // P    # 32 k-tiles over hidden dim
KQ = 8           # k-tiles per xT chunk
NKQ = KT // KQ   # 4 chunks
SBLK = S // 512  # 4 blocks of 512 along sequence
ST = S // P      # 16 s-tiles of 128
SCALING = 128 ** -0.5
EXP_BIAS = -2.0  # constant shift inside exp; cancels in normalization


def _build_nc():
    import concourse.bass as bass
    import concourse.mybir as mybir
    import concourse.tile as tile
    from concourse import bacc
    from concourse import bass_isa
    from concourse.masks import make_identity

    f16 = mybir.dt.float16
    f32 = mybir.dt.float32
    Alu = mybir.AluOpType
    Act = mybir.ActivationFunctionType

    nc = bacc.Bacc(
        "TRN2",
        target_bir_lowering=False,
        debug=False,
        enable_asserts=False,
        num_devices=NCORES,
    )

    xT_d = nc.dram_tensor("xT", [P, SBLK, KT, 512], f16, kind="ExternalInput")
    wq_d = nc.dram_tensor("wq", [P, KT, HPC * P], f16, kind="ExternalInput")
    wk_d = nc.dram_tensor("wk", [P, KT, P], f16, kind="ExternalInput")
    wv_d = nc.dram_tensor("wv", [P, KT, P], f16, kind="ExternalInput")
    wo_d = nc.dram_tensor("wo", [P, HPC, HID], f16, kind="ExternalInput")
    cos_d = nc.dram_tensor("cosT", [P, S], f16, kind="ExternalInput")
    sin_d = nc.dram_tensor("sinS", [P, S], f16, kind="ExternalInput")
    tri_d = nc.dram_tensor("tri", [P, P], f16, kind="ExternalInput")
    out_d = nc.dram_tensor("out", [S, HID], f16, kind="ExternalOutput")

    with tile.TileContext(nc) as tc:
        with tc.tile_pool(name="const", bufs=1) as constp:
            qTs = [[constp.tile([P, 512], f16, tag=f"qT{h}_{sb}", name=f"qT{h}_{sb}")
                    for sb in range(SBLK)] for h in range(HPC)]
            kTs = [constp.tile([P, 512], f16, tag=f"kT{sb}", name=f"kT{sb}")
                   for sb in range(SBLK)]
            Vts = [constp.tile([P, P], f16, tag=f"Vt{st}", name=f"Vt{st}")
                   for st in range(ST)]
            OTs = [constp.tile([P, 512], f16, tag=f"OT{i}", name=f"OT{i}")
                   for i in range(HPC * SBLK)]
            ident = constp.tile([P, P], f16, tag="ident")
            make_identity(nc, ident[:])
            ebias = constp.tile([P, 1], f32, tag="ebias")
            nc.vector.memset(ebias[:], EXP_BIAS)
            ones16 = constp.tile([P, 1], f16, tag="o16")
            nc.vector.memset(ones16[:], 1.0)
            ones_row = constp.tile([1, P], f32, tag="orow")
            nc.vector.memset(ones_row[:], 1.0)
            cos_sb = constp.tile([P, S], f16, tag="cos")
            sin_sb = constp.tile([P, S], f16, tag="sin")
            tri_sb = constp.tile([P, P], f16, tag="tri")
            wo_sb = constp.tile([P, HPC, HID], f16, tag="wo")

            # ---- phase 1: Q/K/V projections (+ RoPE on q, k) ----
            with (
                tc.tile_pool(name="p1w", bufs=1) as p1w,
                tc.tile_pool(name="xt", bufs=4) as xtp,
                tc.tile_pool(name="rope", bufs=3) as ropep,
                tc.tile_pool(name="ps1", bufs=1, space="PSUM") as ps1,
            ):
                # PSUM bank assignment order matters: phase 2's ps_s lands on
                # the first banks allocated here, so allocate the chains that
                # are read earliest at the phase boundary (k, v, transposes)
                # first, and rope in matching order (k first, q0 last; q0 is
                # double-buffered for the sb-boundary handoff).
                ps_warm = ps1.tile([P, 512], f32, tag="psvt", name="warm")
                for _ in range(24):
                    nc.tensor.matmul(ps_warm[:, 0:P], ident[:], ident[:],
                                     start=True, stop=True)

                # startup-critical loads split in halves across four queues;
                # everything else (cos/sin/tri/wo/wq1-3) deferred to kq=1
                wq_sb = [p1w.tile([P, KQ, HPC * P], f16, tag=f"wq{kq}", name=f"wq{kq}")
                         for kq in range(NKQ)]
                nc.scalar.dma_start(wq_sb[0][:, 0:KQ // 2, :], wq_d.ap()[:, 0:KQ // 2, :])
                nc.scalar.dma_start(
                    wq_sb[0][:, KQ // 2:KQ, :], wq_d.ap()[:, KQ // 2:KQ, :]
                )
                wk_sb = p1w.tile([P, KT, P], f16, tag="wk")
                wv_sb = p1w.tile([P, KT, P], f16, tag="wv")
                HK = KT // 2
                nc.gpsimd.dma_start(wk_sb[:, 0:HK, :], wk_d.ap()[:, 0:HK, :])
                nc.gpsimd.dma_start(wv_sb[:, 0:HK, :], wv_d.ap()[:, 0:HK, :])

                def rope(p16, out, scol):
                    """out(f16) = p16*cos + rotate_half(p16)*sin; the sign of
                    rotate_half is folded into sinS on the host. p16 is the
                    fp16 SBUF copy of the projection (the PSUM bank was
                    already released by the ScalarE evacuation copy)."""
                    c = cos_sb[:, scol]
                    sn = sin_sb[:, scol]
                    t1 = ropep.tile([P, 512], f16, tag="t1")
                    nc.vector.tensor_tensor(t1[:], p16[:], c, Alu.mult)
                    t2 = ropep.tile([P, 512], f16, tag="t2")
                    # rotate_half: lanes 0:64 read partitions 64:128, vice
                    # versa; sinS is pre-rolled by 64 partitions on the host so
                    # both SBUF operands share a base partition (NCC_IBIR297)
                    nc.vector.tensor_tensor(t2[0:64, :], p16[64:128, :], sn[64:128, :], Alu.mult)
                    nc.vector.tensor_tensor(t2[64:128, :], p16[0:64, :], sn[0:64, :], Alu.mult)
                    nc.vector.tensor_tensor(out, t1[:], t2[:], Alu.add)

                for sb in range(SBLK):
                    scol = slice(sb * 512, (sb + 1) * 512)
                    # allocation (=bank) order: psvt, pstr(warm above), psq0..:
                    # phase 2's ps_s reuses the first 4 banks, which are the
                    # ones freed soonest after sb=3 (vts copy, transposes,
                    # psq0's idle second buffer, rope q0)
                    ps_vt = ps1.tile([P, 512], f32, tag="psvt")
                    ps_q = [ps1.tile([P, 512], f32, tag=f"psq{h}", name=f"ps_q{h}",
                                     bufs=(2 if h == 0 else 1))
                            for h in range(HPC)]
                    ps_k = ps1.tile([P, 512], f32, tag="psk")
                    for kq in range(NKQ):
                        # xt chunks alternate sync/gpsimd queues; each queue
                        # tops out ~130GB/s, and the early loads nearly
                        # saturate aggregate HBM, so just-in-time order matters
                        xq = nc.sync if kq % 2 == 0 else nc.gpsimd
                        xt = xtp.tile([P, KQ, 512], f16, tag="xt", name=f"xt{kq}")
                        if sb == 0 and kq <= 1:
                            # first two chunks split in halves across both
                            # queues: the startup is aggregate-HBM-bound and
                            # every early packet matters
                            nc.sync.dma_start(
                                xt[:, 0:KQ // 2, :],
                                xT_d.ap()[:, sb, kq * KQ:kq * KQ + KQ // 2, :],
                            )
                            nc.gpsimd.dma_start(
                                xt[:, KQ // 2:KQ, :],
                                xT_d.ap()[:, sb, kq * KQ + KQ // 2:(kq + 1) * KQ, :],
                            )
                        else:
                            xq.dma_start(
                                xt[:], xT_d.ap()[:, sb, kq * KQ:(kq + 1) * KQ, :]
                            )
                        if sb == 0 and kq == 1:
                            # second halves of wk/wv land just before k-tile 16
                            nc.gpsimd.dma_start(wk_sb[:, HK:KT, :], wk_d.ap()[:, HK:KT, :])
                            nc.gpsimd.dma_start(wv_sb[:, HK:KT, :], wv_d.ap()[:, HK:KT, :])
                        if sb == 0 and kq == 3:
                            # scalar queue is idle after the wq chunks
                            nc.scalar.dma_start(cos_sb[:], cos_d.ap())
                            nc.scalar.dma_start(sin_sb[:], sin_d.ap())
                            nc.scalar.dma_start(tri_sb[:], tri_d.ap())
                        if sb == 1 and kq == 0:
                            # deferred past sb=0's HBM crunch; needed ~170us out
                            nc.scalar.dma_start(wo_sb[:], wo_d.ap())
                        if sb == 0 and kq < NKQ - 1:
                            nc.scalar.dma_start(
                                wq_sb[kq + 1][:], wq_d.ap()[:, (kq + 1) * KQ:(kq + 2) * KQ, :]
                            )
                        for k in range(KQ):
                            kg = kq * KQ + k
                            st, sp = kg == 0, kg == KT - 1
                            for h in range(HPC):
                                nc.tensor.matmul(
                                    ps_q[h][:],
                                    wq_sb[kq][:, k, h * P:(h + 1) * P],
                                    xt[:, k, :],
                                    start=st, stop=sp,
                                )
                            nc.tensor.matmul(
                                ps_k[:], wk_sb[:, kg, :], xt[:, k, :],
                                start=st, stop=sp,
                            )
                            nc.tensor.matmul(
                                ps_vt[:], wv_sb[:, kg, :], xt[:, k, :],
                                start=st, stop=sp,
                            )
                    # Evacuate all six PSUM chains to fp16 SBUF on the
                    # ScalarE (idle in phase 1): frees the PSUM banks within
                    # ~4us of the last matmul instead of after the full DVE
                    # rope chain (~12us), so the next sb / phase 2 start fast.
                    vts = ropep.tile([P, 512], f16, tag="vts")
                    nc.scalar.copy(vts[:], ps_vt[:])
                    q16 = [ropep.tile([P, 512], f16, tag=f"q16_{h}", name=f"q16_{h}")
                           for h in range(HPC)]
                    for h in range(HPC):
                        nc.scalar.copy(q16[h][:], ps_q[h][:])
                    k16 = ropep.tile([P, 512], f16, tag="k16")
                    nc.scalar.copy(k16[:], ps_k[:])
                    # V^T [hd, s] -> V [s, hd] via PE transpose, ahead of the
                    # ropes in the PE FIFO (only waits on the vts copy)
                    for j in range(4):
                        ps_tr = ps1.tile([P, P], f16, tag="pstr")
                        nc.tensor.transpose(ps_tr[:], vts[:, j * P:(j + 1) * P], ident[:])
                        nc.vector.tensor_copy(Vts[sb * 4 + j][:], ps_tr[:])
                    for h in range(HPC):
                        rope(q16[h], qTs[h][sb][:], scol)
                    rope(k16, kTs[sb][:], scol)

            # ---- phase 2+3: attention fused with the output projection ----
            # Block (qb, h) covers q rows [qb*512,(qb+1)*512) of head h.
            # Per block, "units" of 2 k-tiles each: off-diagonal units are
            # full-width; the 2 diagonal units narrow each 128-wide k-tile's
            # matmul/exp/attnV to q columns >= its offset (causal).
            with (
                tc.tile_pool(name="exp", bufs=16) as expp,
                tc.tile_pool(name="norm", bufs=2) as normp,
                tc.tile_pool(name="stage", bufs=3) as stagep,
                tc.tile_pool(name="ps2", bufs=1, space="PSUM") as ps2,
            ):
                GORDER = [0, 1, 2, 3]
                blocks = [(qb, h) for qb in GORDER for h in range(HPC)]
                NBLK = len(blocks)

                pending = []          # normalize finishes deferred one block
                wo_queue = []         # (st, nt) output-projection chains
                stage_tiles = {}      # st -> [tile, ndone]
                copy_rr = [0]
                released = [0]        # count of groups whose wo chains are queued

                def finish(ent):
                    # bc16 (1/colsum broadcast over partitions) was staged to
                    # SBUF a block ago; this is a single ready-to-run DVE op
                    oti, ps_o, bc16 = ent
                    nc.vector.tensor_tensor(OTs[oti][:], ps_o[:], bc16[:], Alu.mult)

                def emit_wo_chain():
                    if not wo_queue:
                        return
                    st, nt = wo_queue.pop(0)
                    if st not in stage_tiles:
                        stage_tiles[st] = [
                            stagep.tile([P, HID], f16, tag="stage", name="stage"), 0
                        ]
                    stage, _ = stage_tiles[st]
                    ssl = slice((st % 4) * P, (st % 4 + 1) * P)
                    ps_w = ps2.tile([P, 512], f32, tag="ps_w", name="ps_w", bufs=2)
                    for h in range(HPC):
                        nc.tensor.matmul(
                            ps_w[:],
                            OTs[h * SBLK + st // 4][:, ssl],
                            wo_sb[:, h, nt * 512:(nt + 1) * 512],
                            start=(h == 0),
                            stop=(h == HPC - 1),
                        )
                    dst = stage[:, nt * 512:(nt + 1) * 512]
                    copy_rr[0] += 1
                    if copy_rr[0] % 2 == 0:  # 1:1 ACT:DVE balance
                        nc.scalar.copy(dst, ps_w[:])
                    else:
                        nc.vector.tensor_copy(dst, ps_w[:])
                    stage_tiles[st][1] += 1
                    if stage_tiles[st][1] == HID // 512:
                        nc.sync.dma_start(out_d.ap()[st * P:(st + 1) * P, :], stage[:])
                        del stage_tiles[st]

                def emit_unit(h, qb, g, acc):
                    """Unit g of block (qb,h): QK matmuls + exp + mask + acc
                    for k-tiles kb = 2g, 2g+1. Off-diagonal units (kb < 4qb)
                    are full-width single-exp; diagonal units narrow each
                    tile to q cols >= 128*j (j = kb - 4qb) and use the
                    shared [P,128] lower-triangle mask on the leading strip.
                    Returns (ex_tile, [(kb, exslice, width, coloff)])."""
                    ps_s = ps2.tile([P, 1024], f32, tag="ps_s", name="ps_s", bufs=2)
                    ex = expp.tile([P, 1024], f16, tag="ex", name="ex")
                    parts = []
                    diag = 2 * g >= 4 * qb
                    for t in range(2):
                        kb = 2 * g + t
                        j = kb - 4 * qb
                        off = j * P if diag else 0
                        w = 512 - off
                        sl = slice(t * 512 + off, (t + 1) * 512)
                        nc.tensor.matmul(
                            ps_s[:, sl],
                            kTs[kb // 4][:, (kb % 4) * P:(kb % 4 + 1) * P],
                            qTs[h][qb][:, off:512],
                            start=True,
                            stop=True,
                        )
                        parts.append((kb, sl, w, off))
                    if not diag:
                        nc.scalar.activation(
                            ex[:], ps_s[:], Act.Exp, bias=ebias[:], scale=SCALING
                        )
                    else:
                        for kb, sl, w, off in parts:
                            nc.scalar.activation(
                                ex[:, sl], ps_s[:, sl], Act.Exp,
                                bias=ebias[:], scale=SCALING,
                            )
                    for kb, sl, w, off in parts:
                        half = ex[:, sl]
                        if diag:
                            strip = ex[:, sl.start:sl.start + P]
                            nc.vector.tensor_tensor(strip, strip, tri_sb[:], Alu.mult)
                        if kb == 0:
                            nc.vector.tensor_copy(acc[:], half)
                        else:
                            nc.vector.tensor_tensor(
                                acc[:, off:512], acc[:, off:512], half, Alu.add
                            )
                    return ex, parts

                prev = None  # (units, nunit, ps_o, acc, oti)
                for i in range(NBLK + 1):
                    # wo chains for q-block group qb become eligible two
                    # blocks after the group's last block (normalize is
                    # deferred one block)
                    if i >= 6 and (i - 6) % 4 == 0:
                        qb_ready = GORDER[(i - 6) // 4]
                        released[0] = (i - 6) // 4 + 1
                        for st in range(qb_ready * 4, qb_ready * 4 + 4):
                            for nt in range(HID // 512):
                                wo_queue.append((st, nt))
                    if i < NBLK:
                        qb, h = blocks[i]
                        nunit = 2 * qb + 2
                        units = []
                        acc = normp.tile([P, 512], f16, tag="acc", name="acc", bufs=2)
                    else:
                        nunit = 0
                    if prev is not None:
                        punits, pnunit, ps_o, pacc, oti = prev
                    else:
                        pnunit = 0
                    for g in range(max(nunit, pnunit)):
                        if i < NBLK and g < nunit:
                            units.append(emit_unit(h, qb, g, acc))
                        if prev is not None and g < pnunit:
                            pex, pparts = punits[g]
                            last_kb = 2 * pnunit - 1
                            for kb, sl, w, off in pparts:
                                nc.tensor.matmul(
                                    ps_o[:, off:512], Vts[kb][:], pex[:, sl],
                                    start=(kb == 0), stop=(kb == last_kb),
                                )
                        # hold chains back near the end so the PE has wo work
                        # while the last blocks' normalizes complete
                        for _ in range(1 if i >= NBLK - 2 else 2):
                            emit_wo_chain()
                    if prev is not None:
                        # denominator: ones-vector matmul reduces acc over the
                        # partition axis (borrowing a ps_w rotation slot), a
                        # K=1 ones-row matmul broadcasts 1/colsum back across
                        # all 128 partitions, and the ScalarE stages it to
                        # SBUF — no DMA round trip anywhere on this path
                        ps_cs = ps2.tile([P, 512], f32, tag="ps_w", name="ps_cs",
                                         bufs=2)
                        nc.tensor.matmul(ps_cs[0:1, :], ones16[:], pacc[:],
                                         start=True, stop=True)
                        rec = normp.tile([1, 512], f32, tag="rec", name="rec",
                                         bufs=2)
                        nc.vector.reciprocal_approx_fast(rec[:], ps_cs[0:1, :])
                        ps_bc = ps2.tile([P, 512], f32, tag="ps_w", name="ps_bc",
                                         bufs=2)
                        nc.tensor.matmul(ps_bc[:], ones_row[:], rec[:],
                                         start=True, stop=True)
                        bc16 = normp.tile([P, 512], f32, tag="bc", name="bc16",
                                          bufs=2)
                        nc.scalar.copy(bc16[:], ps_bc[:])
                        if pending:
                            finish(pending.pop())
                        pending.append((oti, ps_o, bc16))
                    if i < NBLK:
                        ps_o = ps2.tile([P, 512], f32, tag="ps_o", name="ps_o", bufs=2)
                        prev = (units, nunit, ps_o, acc, h * SBLK + qb)
                while pending:
                    finish(pending.pop())
                for j in range(released[0], SBLK):
                    qb = GORDER[j]
                    for st in range(qb * 4, qb * 4 + 4):
                        for nt in range(HID // 512):
                            wo_queue.append((st, nt))
                while wo_queue:
                    emit_wo_chain()

    nc.compile()
    return nc


_CACHE = {}


def _get_nc():
    if "nc" not in _CACHE:
        _CACHE["nc"] = _build_nc()
    return _CACHE["nc"]


def make_in_maps(hidden_states, cos, sin, wq, wk, wv, wo):
    x = np.asarray(hidden_states)[0].astype(np.float16)  # [S, HID]
    # xT_pre[p, sb, kt, c] = x[sb*512+c, kt*128+p]
    xT_pre = np.ascontiguousarray(
        x.reshape(SBLK, 512, KT, P).transpose(3, 0, 2, 1)
    )
    cosT = np.ascontiguousarray(np.asarray(cos)[0].T).astype(np.float16)
    sinT = np.ascontiguousarray(np.asarray(sin)[0].T).astype(np.float64)
    sinS = sinT.copy()
    sinS[:64] *= -1.0  # rotate_half sign fold: q'[d<64] -= q[d+64]*sin[d]
    # roll by 64 partitions: the kernel reads sn[64:128] against p16[64:128]
    # (and vice versa) to satisfy the equal-base-partition constraint
    sinS = np.concatenate([sinS[64:], sinS[:64]], axis=0)
    sinS = np.ascontiguousarray(sinS).astype(np.float16)
    f = np.arange(P)[None, :]
    p = np.arange(P)[:, None]
    tri = (f >= p).astype(np.float16)  # [128,128] lower-tri in col>=row sense
    wq = np.asarray(wq).astype(np.float16)
    wk = np.asarray(wk).astype(np.float16)
    wv = np.asarray(wv).astype(np.float16)
    wo = np.asarray(wo).astype(np.float16)

    in_maps = []
    for c in range(NCORES):
        wq_c = wq[:, c * 512:(c + 1) * 512]    # [4096, 512]
        wk_c = wk[:, c * P:(c + 1) * P]        # [4096, 128]
        wv_c = wv[:, c * P:(c + 1) * P]
        wo_c = wo[c * 512:(c + 1) * 512, :]    # [512, 4096]
        in_maps.append(
            {
                "xT": xT_pre,
                "wq": np.ascontiguousarray(
                    wq_c.reshape(KT, P, 512).transpose(1, 0, 2)
                ),
                "wk": np.ascontiguousarray(
                    wk_c.reshape(KT, P, P).transpose(1, 0, 2)
                ),
                "wv": np.ascontiguousarray(
                    wv_c.reshape(KT, P, P).transpose(1, 0, 2)
                ),
                "wo": np.ascontiguousarray(
                    wo_c.reshape(HPC, P, HID).transpose(1, 0, 2)
                ),
                "cosT": cosT,
                "sinS": sinS,
                "tri": tri,
            }
        )
    return in_maps


def run(in_maps, trace=False, **kw):
    from concourse.bass_utils import run_bass_kernel_spmd

    nc = _get_nc()
    return run_bass_kernel_spmd(
        nc, in_maps, core_ids=list(range(NCORES)), trace=trace, **kw
    )


def kernel(hidden_states, cos, sin, attn_mask, wq, wk, wv, wo):
    in_maps = make_in_maps(hidden_states, cos, sin, wq, wk, wv, wo)
    res = run(in_maps)
    parts = np.stack([np.asarray(r["out"], dtype=np.float32) for r in res.results])
    out = parts.sum(axis=0, dtype=np.float64).astype(np.float32)
    return out.reshape(1, S, HID)


# revision 47
# speedup vs baseline: 1.0666x; 1.0522x over previous
"""Trainium2 Bass kernel for Llama GQA attention (B=1, S=2048, HID=4096,
NH=32, NKV=8, HD=128), tensor-parallel over 8 NeuronCores.

Core c owns Q heads [4c, 4c+4) and KV head c (one GQA group per core).
Each core computes its partial contribution to out = attn_out @ wo (wo is
sharded on its input dim); the host sums the 8 partials.

v2 over the 472us baseline:
- all DRAM inputs pre-laid-out on the host so every DMA reads >=8KB
  contiguous per partition (was 256B-1KB packets; startup was DMA-bound)
- phase-2 weights (wo, mask) + cos/sin loaded during phase 1; phase-2
  SBUF pools reuse phase-1 space with no pool-exit barrier on the PE path
- ps_s (QK scores PSUM) double-buffered so block i+1's QK matmuls overlap
  block i's exp on the ScalarE (was a 1.2us serialization per pair)
- causal narrowing: diagonal 128-col k-tiles only compute q columns
  >= tile offset (saves ~37% of diagonal QK/exp/attnV work)
- softmax denominator via GpSimd partition-reduce + partition-broadcast
  (was: PE ones-matmul + round-trip DMA bounce through DRAM)
- output written fp16 (host sums partials in fp32)
"""

import sys

sys.path.insert(0, "/opt/trn_rl_repo")

import numpy as np

P = 128
S = 2048
HID = 4096
HPC = 4          # q heads per core
NCORES = 8
KT = HID // P    # 32 k-tiles over hidden dim
KQ = 8           # k-tiles per xT chunk
NKQ = KT // KQ   # 4 chunks
SBLK = S // 512  # 4 blocks of 512 along sequence
ST = S // P      # 16 s-tiles of 128
SCALING = 128 ** -0.5
EXP_BIAS = -2.0  # constant shift inside exp; cancels in normalization


def _build_nc():
    import concourse.bass as bass
    import concourse.mybir as mybir
    import concourse.tile as tile
    from concourse import bacc
    from concourse import bass_isa
    from concourse.masks import make_identity

    f16 = mybir.dt.float16
    f32 = mybir.dt.float32
    Alu = mybir.AluOpType
    Act = mybir.ActivationFunctionType

    nc = bacc.Bacc(
        "TRN2",
        target_bir_lowering=False,
        debug=False,
        enable_asserts=False,
        num_devices=NCORES,
    )

    xT_d = nc.dram_tensor("xT", [P, SBLK, KT, 512], f16, kind="ExternalInput")
    wq_d = nc.dram_tensor("wq", [P, KT, HPC * P], f16, kind="ExternalInput")
    wk_d = nc.dram_tensor("wk", [P, KT, P], f16, kind="ExternalInput")
    wv_d = nc.dram_tensor("wv", [P, KT, P], f16, kind="ExternalInput")
    wo_d = nc.dram_tensor("wo", [P, HPC, HID], f16, kind="ExternalInput")
    cos_d = nc.dram_tensor("cosT", [P, S], f16, kind="ExternalInput")
    sin_d = nc.dram_tensor("sinS", [P, S], f16, kind="ExternalInput")
    tri_d = nc.dram_tensor("tri", [P, P], f16, kind="ExternalInput")
    out_d = nc.dram_tensor("out", [S, HID], f16, kind="ExternalOutput")

    with tile.TileContext(nc) as tc:
        with tc.tile_pool(name="const", bufs=1) as constp:
            qTs = [[constp.tile([P, 512], f16, tag=f"qT{h}_{sb}", name=f"qT{h}_{sb}")
                    for sb in range(SBLK)] for h in range(HPC)]
            kTs = [constp.tile([P, 512], f16, tag=f"kT{sb}", name=f"kT{sb}")
                   for sb in range(SBLK)]
            Vts = [constp.tile([P, P], f16, tag=f"Vt{st}", name=f"Vt{st}")
                   for st in range(ST)]
            OTs = [constp.tile([P, 512], f16, tag=f"OT{i}", name=f"OT{i}")
                   for i in range(HPC * SBLK)]
            ident = constp.tile([P, P], f16, tag="ident")
            make_identity(nc, ident[:])
            ebias = constp.tile([P, 1], f32, tag="ebias")
            nc.vector.memset(ebias[:], EXP_BIAS)
            ones16 = constp.tile([P, 1], f16, tag="o16")
            nc.vector.memset(ones16[:], 1.0)
            ones_row = constp.tile([1, P], f16, tag="orow")
            nc.vector.memset(ones_row[:], 1.0)
            cos_sb = constp.tile([P, S], f16, tag="cos")
            sin_sb = constp.tile([P, S], f16, tag="sin")
            tri_sb = constp.tile([P, P], f16, tag="tri")
            wo_sb = constp.tile([P, HPC, HID], f16, tag="wo")

            # ---- phase 1: Q/K/V projections (+ RoPE on q, k) ----
            with (
                tc.tile_pool(name="p1w", bufs=1) as p1w,
                tc.tile_pool(name="xt", bufs=4) as xtp,
                tc.tile_pool(name="rope", bufs=3) as ropep,
                tc.tile_pool(name="ps1", bufs=1, space="PSUM") as ps1,
            ):
                # PSUM bank assignment order matters: phase 2's ps_s lands on
                # the first banks allocated here, so allocate the chains that
                # are read earliest at the phase boundary (k, v, transposes)
                # first, and rope in matching order (k first, q0 last; q0 is
                # double-buffered for the sb-boundary handoff).
                ps_warm = ps1.tile([P, 512], f32, tag="psvt", name="warm")
                for _ in range(24):
                    nc.tensor.matmul(ps_warm[:, 0:P], ident[:], ident[:],
                                     start=True, stop=True)

                # startup-critical loads split in halves across four queues;
                # everything else (cos/sin/tri/wo/wq1-3) deferred to kq=1
                wq_sb = [p1w.tile([P, KQ, HPC * P], f16, tag=f"wq{kq}", name=f"wq{kq}")
                         for kq in range(NKQ)]
                nc.scalar.dma_start(wq_sb[0][:, 0:KQ // 2, :], wq_d.ap()[:, 0:KQ // 2, :])
                nc.scalar.dma_start(
                    wq_sb[0][:, KQ // 2:KQ, :], wq_d.ap()[:, KQ // 2:KQ, :]
                )
                wk_sb = p1w.tile([P, KT, P], f16, tag="wk")
                wv_sb = p1w.tile([P, KT, P], f16, tag="wv")
                HK = KT // 2

                def rope(p16, out, scol):
                    """out(f16) = p16*cos + rotate_half(p16)*sin; the sign of
                    rotate_half is folded into sinS on the host. p16 is the
                    fp16 SBUF copy of the projection (the PSUM bank was
                    already released by the ScalarE evacuation copy)."""
                    c = cos_sb[:, scol]
                    sn = sin_sb[:, scol]
                    t1 = ropep.tile([P, 512], f16, tag="t1")
                    nc.vector.tensor_tensor(t1[:], p16[:], c, Alu.mult)
                    t2 = ropep.tile([P, 512], f16, tag="t2")
                    # rotate_half: lanes 0:64 read partitions 64:128, vice
                    # versa; sinS is pre-rolled by 64 partitions on the host so
                    # both SBUF operands share a base partition (NCC_IBIR297)
                    nc.vector.tensor_tensor(t2[0:64, :], p16[64:128, :], sn[64:128, :], Alu.mult)
                    nc.vector.tensor_tensor(t2[64:128, :], p16[0:64, :], sn[0:64, :], Alu.mult)
                    nc.vector.tensor_tensor(out, t1[:], t2[:], Alu.add)

                for sb in range(SBLK):
                    scol = slice(sb * 512, (sb + 1) * 512)
                    # allocation (=bank) order: psvt, pstr(warm above), psq0..:
                    # phase 2's ps_s reuses the first 4 banks, which are the
                    # ones freed soonest after sb=3 (vts copy, transposes,
                    # psq0's idle second buffer, rope q0)
                    ps_vt = ps1.tile([P, 512], f32, tag="psvt")
                    ps_q = [ps1.tile([P, 512], f32, tag=f"psq{h}", name=f"ps_q{h}",
                                     bufs=(2 if h == 0 else 1))
                            for h in range(HPC)]
                    ps_k = ps1.tile([P, 512], f32, tag="psk")

                    xts = []

                    def kv_pass(kqq):
                        for k in range(KQ):
                            kg = kqq * KQ + k
                            nc.tensor.matmul(
                                ps_k[:], wk_sb[:, kg, :], xts[kqq][:, k, :],
                                start=kg == 0, stop=kg == KT - 1,
                            )
                        for k in range(KQ):
                            kg = kqq * KQ + k
                            nc.tensor.matmul(
                                ps_vt[:], wv_sb[:, kg, :], xts[kqq][:, k, :],
                                start=kg == 0, stop=kg == KT - 1,
                            )

                    for kq in range(NKQ):
                        # xt chunks alternate sync/gpsimd queues; each queue
                        # tops out ~130GB/s, and the early loads nearly
                        # saturate aggregate HBM, so just-in-time order matters
                        xq = nc.sync if kq % 2 == 0 else nc.gpsimd
                        xt = xtp.tile([P, KQ, 512], f16, tag="xt", name=f"xt{kq}")
                        if sb == 0 and kq <= 1:
                            # first two chunks split in halves across both
                            # queues: the startup is aggregate-HBM-bound and
                            # every early packet matters
                            nc.sync.dma_start(
                                xt[:, 0:KQ // 2, :],
                                xT_d.ap()[:, sb, kq * KQ:kq * KQ + KQ // 2, :],
                            )
                            nc.gpsimd.dma_start(
                                xt[:, KQ // 2:KQ, :],
                                xT_d.ap()[:, sb, kq * KQ + KQ // 2:(kq + 1) * KQ, :],
                            )
                        else:
                            xq.dma_start(
                                xt[:], xT_d.ap()[:, sb, kq * KQ:(kq + 1) * KQ, :]
                            )
                        if sb == 0 and kq == 1:
                            # wk/wv after the xt chunks that gate sooner; the
                            # K/V matmuls run one chunk behind the Q matmuls
                            nc.gpsimd.dma_start(wk_sb[:, 0:HK, :], wk_d.ap()[:, 0:HK, :])
                            nc.gpsimd.dma_start(wv_sb[:, 0:HK, :], wv_d.ap()[:, 0:HK, :])
                            nc.gpsimd.dma_start(wk_sb[:, HK:KT, :], wk_d.ap()[:, HK:KT, :])
                            nc.gpsimd.dma_start(wv_sb[:, HK:KT, :], wv_d.ap()[:, HK:KT, :])
                        if sb == 0 and kq == 3:
                            # scalar queue is idle after the wq chunks
                            nc.scalar.dma_start(cos_sb[:], cos_d.ap())
                            nc.scalar.dma_start(sin_sb[:], sin_d.ap())
                            nc.scalar.dma_start(tri_sb[:], tri_d.ap())
                        if sb == 1 and kq == 0:
                            # deferred past sb=0's HBM crunch; needed ~170us out
                            nc.scalar.dma_start(wo_sb[:], wo_d.ap())
                        if sb == 0 and kq < NKQ - 1:
                            nc.scalar.dma_start(
                                wq_sb[kq + 1][:], wq_d.ap()[:, (kq + 1) * KQ:(kq + 2) * KQ, :]
                            )
                        for k in range(KQ):
                            kg = kq * KQ + k
                            st, sp = kg == 0, kg == KT - 1
                            for h in range(HPC):
                                nc.tensor.matmul(
                                    ps_q[h][:],
                                    wq_sb[kq][:, k, h * P:(h + 1) * P],
                                    xt[:, k, :],
                                    start=st, stop=sp,
                                )
                        # K/V chains run one chunk behind Q: when the next
                        # chunk's wq/xt DMA is late, the PE FIFO still holds
                        # this ready work instead of stalling (and cooling)
                        xts.append(xt)
                        if kq > 0:
                            kv_pass(kq - 1)
                    kv_pass(NKQ - 1)
                    xts.clear()
                    # Evacuate all six PSUM chains to fp16 SBUF on the
                    # ScalarE (idle in phase 1): frees the PSUM banks within
                    # ~4us of the last matmul instead of after the full DVE
                    # rope chain (~12us), so the next sb / phase 2 start fast.
                    vts = ropep.tile([P, 512], f16, tag="vts")
                    nc.scalar.copy(vts[:], ps_vt[:])
                    q16 = [ropep.tile([P, 512], f16, tag=f"q16_{h}", name=f"q16_{h}")
                           for h in range(HPC)]
                    for h in range(HPC):
                        nc.scalar.copy(q16[h][:], ps_q[h][:])
                    k16 = ropep.tile([P, 512], f16, tag="k16")
                    nc.scalar.copy(k16[:], ps_k[:])
                    # V^T [hd, s] -> V [s, hd] via PE transpose, ahead of the
                    # ropes in the PE FIFO (only waits on the vts copy)
                    for j in range(4):
                        ps_tr = ps1.tile([P, P], f16, tag="pstr")
                        nc.tensor.transpose(ps_tr[:], vts[:, j * P:(j + 1) * P], ident[:])
                        nc.vector.tensor_copy(Vts[sb * 4 + j][:], ps_tr[:])
                    for h in range(HPC):
                        rope(q16[h], qTs[h][sb][:], scol)
                    rope(k16, kTs[sb][:], scol)

            # ---- phase 2+3: attention fused with the output projection ----
            # Block (qb, h) covers q rows [qb*512,(qb+1)*512) of head h.
            # Per block, "units" of 2 k-tiles each: off-diagonal units are
            # full-width; the 2 diagonal units narrow each 128-wide k-tile's
            # matmul/exp/attnV to q columns >= its offset (causal).
            with (
                tc.tile_pool(name="exp", bufs=16) as expp,
                tc.tile_pool(name="norm", bufs=2) as normp,
                tc.tile_pool(name="stage", bufs=3) as stagep,
                tc.tile_pool(name="ps2", bufs=1, space="PSUM") as ps2,
            ):
                GORDER = [0, 1, 2, 3]
                blocks = [(qb, h) for qb in GORDER for h in range(HPC)]
                NBLK = len(blocks)

                pending = []          # normalize finishes deferred one block
                wo_queue = []         # (st, nt) output-projection chains
                stage_tiles = {}      # st -> [tile, ndone]
                copy_rr = [0]
                released = [0]        # count of groups whose wo chains are queued

                def finish(ent):
                    # bc16 (1/colsum broadcast over partitions) was staged to
                    # SBUF a block ago; this is a single ready-to-run DVE op
                    oti, ps_o, bc16 = ent
                    nc.vector.tensor_tensor(OTs[oti][:], ps_o[:], bc16[:], Alu.mult)

                def emit_wo_chain():
                    if not wo_queue:
                        return
                    st, nt = wo_queue.pop(0)
                    if st not in stage_tiles:
                        stage_tiles[st] = [
                            stagep.tile([P, HID], f16, tag="stage", name="stage"), 0
                        ]
                    stage, _ = stage_tiles[st]
                    ssl = slice((st % 4) * P, (st % 4 + 1) * P)
                    ps_w = ps2.tile([P, 512], f32, tag="ps_w", name="ps_w", bufs=2)
                    for h in range(HPC):
                        nc.tensor.matmul(
                            ps_w[:],
                            OTs[h * SBLK + st // 4][:, ssl],
                            wo_sb[:, h, nt * 512:(nt + 1) * 512],
                            start=(h == 0),
                            stop=(h == HPC - 1),
                        )
                    dst = stage[:, nt * 512:(nt + 1) * 512]
                    copy_rr[0] += 1
                    if copy_rr[0] % 2 == 0:  # 1:1 ACT:DVE balance
                        nc.scalar.copy(dst, ps_w[:])
                    else:
                        nc.vector.tensor_copy(dst, ps_w[:])
                    stage_tiles[st][1] += 1
                    if stage_tiles[st][1] == HID // 512:
                        nc.sync.dma_start(out_d.ap()[st * P:(st + 1) * P, :], stage[:])
                        del stage_tiles[st]

                def emit_unit(h, qb, g, acc):
                    """Unit g of block (qb,h): QK matmuls + exp + mask + acc
                    for k-tiles kb = 2g, 2g+1. Off-diagonal units (kb < 4qb)
                    are full-width single-exp; diagonal units narrow each
                    tile to q cols >= 128*j (j = kb - 4qb) and use the
                    shared [P,128] lower-triangle mask on the leading strip.
                    Returns (ex_tile, [(kb, exslice, width, coloff)])."""
                    ps_s = ps2.tile([P, 1024], f32, tag="ps_s", name="ps_s", bufs=2)
                    ex = expp.tile([P, 1024], f16, tag="ex", name="ex")
                    parts = []
                    diag = 2 * g >= 4 * qb
                    for t in range(2):
                        kb = 2 * g + t
                        j = kb - 4 * qb
                        off = j * P if diag else 0
                        w = 512 - off
                        sl = slice(t * 512 + off, (t + 1) * 512)
                        nc.tensor.matmul(
                            ps_s[:, sl],
                            kTs[kb // 4][:, (kb % 4) * P:(kb % 4 + 1) * P],
                            qTs[h][qb][:, off:512],
                            start=True,
                            stop=True,
                        )
                        parts.append((kb, sl, w, off))
                    if not diag:
                        nc.scalar.activation(
                            ex[:], ps_s[:], Act.Exp, bias=ebias[:], scale=SCALING
                        )
                    else:
                        for kb, sl, w, off in parts:
                            nc.scalar.activation(
                                ex[:, sl], ps_s[:, sl], Act.Exp,
                                bias=ebias[:], scale=SCALING,
                            )
                    for kb, sl, w, off in parts:
                        half = ex[:, sl]
                        if diag:
                            strip = ex[:, sl.start:sl.start + P]
                            nc.vector.tensor_tensor(strip, strip, tri_sb[:], Alu.mult)
                        if kb == 0:
                            nc.vector.tensor_copy(acc[:], half)
                        else:
                            nc.vector.tensor_tensor(
                                acc[:, off:512], acc[:, off:512], half, Alu.add
                            )
                    return ex, parts

                prev = None  # (units, nunit, ps_o, acc, oti)
                for i in range(NBLK + 1):
                    # wo chains for q-block group qb become eligible two
                    # blocks after the group's last block (normalize is
                    # deferred one block)
                    if i >= 6 and (i - 6) % 4 == 0:
                        qb_ready = GORDER[(i - 6) // 4]
                        released[0] = (i - 6) // 4 + 1
                        for st in range(qb_ready * 4, qb_ready * 4 + 4):
                            for nt in range(HID // 512):
                                wo_queue.append((st, nt))
                    if i < NBLK:
                        qb, h = blocks[i]
                        nunit = 2 * qb + 2
                        units = []
                        acc = normp.tile([P, 512], f16, tag="acc", name="acc", bufs=2)
                    else:
                        nunit = 0
                    if prev is not None:
                        punits, pnunit, ps_o, pacc, oti = prev
                    else:
                        pnunit = 0
                    for g in range(max(nunit, pnunit)):
                        if i < NBLK and g < nunit:
                            units.append(emit_unit(h, qb, g, acc))
                        if prev is not None and g < pnunit:
                            pex, pparts = punits[g]
                            last_kb = 2 * pnunit - 1
                            for kb, sl, w, off in pparts:
                                nc.tensor.matmul(
                                    ps_o[:, off:512], Vts[kb][:], pex[:, sl],
                                    start=(kb == 0), stop=(kb == last_kb),
                                )
                        # hold chains back near the end so the PE has wo work
                        # while the last blocks' normalizes complete
                        for _ in range(1 if i >= NBLK - 2 else 2):
                            emit_wo_chain()
                    if prev is not None:
                        # denominator: ones-vector matmul reduces acc over the
                        # partition axis (borrowing a ps_w rotation slot), a
                        # K=1 ones-row matmul broadcasts 1/colsum back across
                        # all 128 partitions, and the ScalarE stages it to
                        # SBUF — no DMA round trip anywhere on this path
                        ps_cs = ps2.tile([P, 512], f32, tag="ps_w", name="ps_cs",
                                         bufs=2)
                        nc.tensor.matmul(ps_cs[0:1, :], ones16[:], pacc[:],
                                         start=True, stop=True)
                        rec = normp.tile([1, 512], f32, tag="rec", name="rec",
                                         bufs=2)
                        nc.vector.reciprocal_approx_fast(rec[:], ps_cs[0:1, :])
                        rec16 = normp.tile([1, 512], f16, tag="rec16",
                                           name="rec16", bufs=2)
                        nc.scalar.copy(rec16[:], rec[:])
                        ps_bc = ps2.tile([P, 512], f32, tag="ps_w", name="ps_bc",
                                         bufs=2)
                        nc.tensor.matmul(ps_bc[:], ones_row[:], rec16[:],
                                         start=True, stop=True)
                        bc16 = normp.tile([P, 512], f32, tag="bc", name="bc16",
                                          bufs=2)
                        nc.scalar.copy(bc16[:], ps_bc[:])
                        if pending:
                            finish(pending.pop())
                        pending.append((oti, ps_o, bc16))
                    if i < NBLK:
                        ps_o = ps2.tile([P, 512], f32, tag="ps_o", name="ps_o", bufs=2)
                        prev = (units, nunit, ps_o, acc, h * SBLK + qb)
                while pending:
                    finish(pending.pop())
                for j in range(released[0], SBLK):
                    qb = GORDER[j]
                    for st in range(qb * 4, qb * 4 + 4):
                        for nt in range(HID // 512):
                            wo_queue.append((st, nt))
                while wo_queue:
                    emit_wo_chain()

    nc.compile()
    return nc


_CACHE = {}


def _get_nc():
    if "nc" not in _CACHE:
        _CACHE["nc"] = _build_nc()
    return _CACHE["nc"]


def make_in_maps(hidden_states, cos, sin, wq, wk, wv, wo):
    x = np.asarray(hidden_states)[0].astype(np.float16)  # [S, HID]
    # xT_pre[p, sb, kt, c] = x[sb*512+c, kt*128+p]
    xT_pre = np.ascontiguousarray(
        x.reshape(SBLK, 512, KT, P).transpose(3, 0, 2, 1)
    )
    cosT = np.ascontiguousarray(np.asarray(cos)[0].T).astype(np.float16)
    sinT = np.ascontiguousarray(np.asarray(sin)[0].T).astype(np.float64)
    sinS = sinT.copy()
    sinS[:64] *= -1.0  # rotate_half sign fold: q'[d<64] -= q[d+64]*sin[d]
    # roll by 64 partitions: the kernel reads sn[64:128] against p16[64:128]
    # (and vice versa) to satisfy the equal-base-partition constraint
    sinS = np.concatenate([sinS[64:], sinS[:64]], axis=0)
    sinS = np.ascontiguousarray(sinS).astype(np.float16)
    f = np.arange(P)[None, :]
    p = np.arange(P)[:, None]
    tri = (f >= p).astype(np.float16)  # [128,128] lower-tri in col>=row sense
    wq = np.asarray(wq).astype(np.float16)
    wk = np.asarray(wk).astype(np.float16)
    wv = np.asarray(wv).astype(np.float16)
    wo = np.asarray(wo).astype(np.float16)

    in_maps = []
    for c in range(NCORES):
        wq_c = wq[:, c * 512:(c + 1) * 512]    # [4096, 512]
        wk_c = wk[:, c * P:(c + 1) * P]        # [4096, 128]
        wv_c = wv[:, c * P:(c + 1) * P]
        wo_c = wo[c * 512:(c + 1) * 512, :]    # [512, 4096]
        in_maps.append(
            {
                "xT": xT_pre,
                "wq": np.ascontiguousarray(
                    wq_c.reshape(KT, P, 512).transpose(1, 0, 2)
                ),
                "wk": np.ascontiguousarray(
                    wk_c.reshape(KT, P, P).transpose(1, 0, 2)
                ),
                "wv": np.ascontiguousarray(
                    wv_c.reshape(KT, P, P).transpose(1, 0, 2)
                ),
                "wo": np.ascontiguousarray(
                    wo_c.reshape(HPC, P, HID).transpose(1, 0, 2)
                ),
                "cosT": cosT,
                "sinS": sinS,
                "tri": tri,
            }
        )
    return in_maps


def run(in_maps, trace=False, **kw):
    from concourse.bass_utils import run_bass_kernel_spmd

    nc = _get_nc()
    return run_bass_kernel_spmd(
        nc, in_maps, core_ids=list(range(NCORES)), trace=trace, **kw
    )


def kernel(hidden_states, cos, sin, attn_mask, wq, wk, wv, wo):
    in_maps = make_in_maps(hidden_states, cos, sin, wq, wk, wv, wo)
    res = run(in_maps)
    parts = np.stack([np.asarray(r["out"], dtype=np.float32) for r in res.results])
    out = parts.sum(axis=0, dtype=np.float64).astype(np.float32)
    return out.reshape(1, S, HID)
